# revision 17
# baseline (speedup 1.0000x reference)
"""Trainium2 Bass kernel for MllamaTextSdpaAttention (GQA + RoPE + causal SDPA).

Tensor-parallel over heads across 8 NeuronCores. Core c owns q-heads
[4c, 4c+4) and kv-head c. Each core computes hidden @ Wq/Wk/Wv slices, RoPE,
causal attention for its heads, and its row-slice of the Wo matmul, yielding
a partial [T, DIM] output (bf16) summed on the host in f32.

v2 changes vs the 350.5us baseline:
- Softmax denominators no longer use PE ones-matmuls (36.9k wasted PE
  columns). exp tiles are accumulated elementwise on DVE into S[k%128, q],
  then one gpsimd partition_all_reduce gives the broadcast rowsum; DVE
  reciprocal + multiply normalize. Frees a PSUM bank (work 5 + ot 3).
- Software pipelining: attention groups of chunk c are interleaved with the
  projection matmul chains of chunk c+1 (generator filler), so PE does not
  stall on the ACT exp cadence inside groups. The last chunk's groups
  interleave with early O-projection tiles.
- Weights are repacked host-side into [128, ...] SBUF-image layouts so DMA
  descriptors are >=1KB contiguous (half the per-descriptor latency), and the
  first wq/hs DMAs are sliced fine so the first matmul starts at ~3.5us.
- hs tiles are 256 tokens (3-buf pool), one shared cos/sin table (the 1/sqrt(d)
  scale is folded into Wq host-side), wo stays resident in SBUF.
"""

import numpy as np
import ml_dtypes

import concourse.bacc as bacc
import concourse.bass as bass
import concourse.mybir as mybir
from concourse import bass_isa
from concourse.tile import TileContext
from concourse import bass_utils

BF16 = mybir.dt.bfloat16
F32 = mybir.dt.float32

B, S, DIM = 2, 1024, 4096
T = B * S                     # 2048 tokens, batch-major
N_HEADS, N_KV = 32, 8
HD = 128                      # head dim == partition count
N_CORES = 8
HL = N_HEADS // N_CORES       # 4 local q-heads per core
KT = DIM // 128               # 32 feature tiles
CH = 512                      # chunk (q-block) width
HCH = 256                     # hs half-chunk tile width
NCHUNK = T // CH
QB = 512
TT = T // 128                 # 16 token tiles
SCALE = 1.0 / float(np.sqrt(HD))

_CACHE: dict = {}


def _build():
    nc = bacc.Bacc("TRN2", target_bir_lowering=False, debug=False,
                   enable_asserts=False)

    hsT = nc.dram_tensor("hsT", [DIM, T], BF16, kind="ExternalInput")
    wq = nc.dram_tensor("wq", [128, HL, KT, HD], BF16, kind="ExternalInput")
    wk = nc.dram_tensor("wk", [128, KT, HD], BF16, kind="ExternalInput")
    wv = nc.dram_tensor("wv", [128, KT, HD], BF16, kind="ExternalInput")
    wo = nc.dram_tensor("wo", [128, HL, DIM], BF16, kind="ExternalInput")
    cos_d = nc.dram_tensor("cos_d", [HD, T], BF16, kind="ExternalInput")
    sin_d = nc.dram_tensor("sin_d", [HD, T], BF16, kind="ExternalInput")
    maskT = nc.dram_tensor("maskT", [128, 128], BF16, kind="ExternalInput")
    ident = nc.dram_tensor("ident", [128, 128], BF16, kind="ExternalInput")
    out = nc.dram_tensor("out", [T, DIM], BF16, kind="ExternalOutput")

    Exp = mybir.ActivationFunctionType.Exp

    with TileContext(nc) as tc:
        with tc.tile_pool(name="consts", bufs=1) as cpool, \
             tc.tile_pool(name="hs", bufs=3) as hpool, \
             tc.tile_pool(name="rope_tmp", bufs=2) as rpool, \
             tc.tile_pool(name="work_ps", bufs=2, space=bass.MemorySpace.PSUM) as wpool, \
             tc.tile_pool(name="ot_ps", bufs=3, space=bass.MemorySpace.PSUM) as otpool, \
             tc.tile_pool(name="et", bufs=6) as epool, \
             tc.tile_pool(name="ssum", bufs=3) as spool, \
             tc.tile_pool(name="rsbc", bufs=2) as rbpool, \
             tc.tile_pool(name="recip", bufs=2) as rcpool, \
             tc.tile_pool(name="out_sb", bufs=6) as xsbpool:

            wq_h = [cpool.tile([128, KT, HD], BF16, tag=f"wq{m}", name=f"wq{m}")
                    for m in range(HL)]
            wk_sb = cpool.tile([128, KT, HD], BF16, tag="wk")
            wv_sb = cpool.tile([128, KT, HD], BF16, tag="wv")
            cos_sb = cpool.tile([128, T], BF16, tag="cos")
            sin_sb = cpool.tile([128, T], BF16, tag="sin")
            maskT_sb = cpool.tile([128, 128], BF16, tag="maskT")
            ident_sb = cpool.tile([128, 128], BF16, tag="ident")
            qt_rot = cpool.tile([128, HL, T], BF16, tag="qt")
            kt_rot = cpool.tile([128, T], BF16, tag="kt")
            v_sb = cpool.tile([128, TT, HD], BF16, tag="v")
            ao = cpool.tile([128, HL, T], BF16, tag="ao")
            wo_sb = cpool.tile([128, HL, DIM], BF16, tag="wo")

            hsT_r = hsT.ap().rearrange("(kt p) t -> p kt t", p=128)
            hs_tiles: dict = {}

            def issue_hs(c, half, slabs=(16, 16)):
                """DMA one [128, KT, HCH] hs half-chunk, split into kt-slabs."""
                t0 = c * CH + half * HCH
                tile = hpool.tile([128, KT, HCH], BF16, tag="hs",
                                  name=f"hs{c}_{half}")
                hs_tiles[(c, half)] = tile
                k0 = 0
                for w in slabs:
                    nc.sync.dma_start(tile[:, k0:k0 + w, :],
                                      hsT_r[:, k0:k0 + w, t0:t0 + HCH])
                    k0 += w
                assert k0 == KT

            def rope(ps, out_ap, t0):
                """out = ps*cos + halfswap(ps)*sin (signs baked into sin)."""
                c_ap = cos_sb[:, t0:t0 + HCH]
                s_ap = sin_sb[:, t0:t0 + HCH]
                t1 = rpool.tile([128, HCH], F32, tag="r1", name="t1")
                t2 = rpool.tile([128, HCH], F32, tag="r2", name="t2")
                nc.vector.tensor_mul(t1, ps, c_ap)
                nc.vector.tensor_mul(t2[0:64, :], ps[64:128, :], s_ap[0:64, :])
                nc.vector.tensor_mul(t2[64:128, :], ps[0:64, :], s_ap[64:128, :])
                nc.vector.tensor_add(out_ap, t1, t2)

            warm_sb = cpool.tile([128, 256], BF16, tag="warm_sb")
            warm_ps = otpool.tile([128, 512], F32, tag="ot", name="warm_ps")

            def dummy(n):
                """Warm-up matmuls on a memset tile: keep PE busy (and its
                p-state ramped) through DMA-paced stretches. Output is
                discarded."""
                for _ in range(n):
                    nc.tensor.matmul(warm_ps[:, 0:256], warm_sb[:, 0:128],
                                     warm_sb, start=True, stop=True)

            def proj_gen(c, warm=None):
                """Projection chains for chunk c (K -> V -> Q per half so the
                cheap weights unlock PE first); yields PE-ns after each
                matmul. warm: dict (half, chain, kt) -> dummy count emitted
                before that matmul (phase-0 DMA pacing)."""
                def pad(half, chain, kt):
                    if warm:
                        dummy(warm.get((half, chain, kt), 0))

                for half in range(2):
                    t0 = c * CH + half * HCH
                    hs = hs_tiles[(c, half)]
                    ps = wpool.tile([128, HCH], F32, tag="work", name="ps_k")
                    for kt in range(KT):
                        pad(half, 'k', kt)
                        nc.tensor.matmul(ps, wk_sb[:, kt, :], hs[:, kt, :],
                                         start=(kt == 0), stop=(kt == KT - 1))
                        yield HCH * 0.4166
                    rope(ps, kt_rot[:, t0:t0 + HCH], t0)
                    for vi in range(HCH // 128):
                        tt = t0 // 128 + vi
                        ps = wpool.tile([128, HD], F32, tag="work", name="ps_v")
                        for kt in range(KT):
                            pad(half, f'v{vi}', kt)
                            nc.tensor.matmul(ps,
                                             hs[:, kt, vi * 128:(vi + 1) * 128],
                                             wv_sb[:, kt, :],
                                             start=(kt == 0), stop=(kt == KT - 1))
                            yield HD * 0.4166
                        nc.scalar.copy(v_sb[:, tt, :], ps)
                    for m in range(HL):
                        ps = wpool.tile([128, HCH], F32, tag="work", name="ps_q")
                        for kt in range(KT):
                            pad(half, f'q{m}', kt)
                            nc.tensor.matmul(ps, wq_h[m][:, kt, :], hs[:, kt, :],
                                             start=(kt == 0), stop=(kt == KT - 1))
                            yield HCH * 0.4166
                        rope(ps, qt_rot[:, m, t0:t0 + HCH], t0)

            def oproj_gen(tts, split_last=False):
                """Output projection tiles; yields PE-ns after each matmul."""
                last = (tts[-1], DIM // 512 - 1)
                for tt in tts:
                    for ni in range(DIM // 512):
                        ps = wpool.tile([128, 512], F32, tag="work", name="ps_o")
                        for kh in range(HL):
                            nc.tensor.matmul(ps, ao[:, kh, tt * 128:(tt + 1) * 128],
                                             wo_sb[:, kh, ni * 512:(ni + 1) * 512],
                                             start=(kh == 0), stop=(kh == HL - 1))
                            yield 512 * 0.4166
                        osb = xsbpool.tile([128, 512], BF16, tag="osb", name="osb")
                        if split_last and (tt, ni) == last:
                            # two half copies/DMAs to shorten the final drain
                            nc.scalar.copy(osb[:, 0:256], ps[:, 0:256])
                            nc.sync.dma_start(
                                out.ap()[tt * 128:(tt + 1) * 128,
                                         ni * 512:ni * 512 + 256], osb[:, 0:256])
                            nc.vector.tensor_copy(osb[:, 256:], ps[:, 256:])
                            nc.sync.dma_start(
                                out.ap()[tt * 128:(tt + 1) * 128,
                                         ni * 512 + 256:(ni + 1) * 512],
                                osb[:, 256:])
                            continue
                        if (tt * 8 + ni) % 2 == 0:
                            nc.scalar.copy(osb, ps)
                        else:
                            nc.vector.tensor_copy(osb, ps)
                        nc.sync.dma_start(
                            out.ap()[tt * 128:(tt + 1) * 128,
                                     ni * 512:(ni + 1) * 512], osb)

            def mk_filler(gen):
                state = {'bank': 0.0, 'done': False}

                def filler(ns):
                    state['bank'] -= ns
                    while state['bank'] < 0 and not state['done']:
                        got = next(gen, None)
                        if got is None:
                            state['done'] = True
                            return
                        state['bank'] += got
                return filler

            def drain(gen):
                for _ in gen:
                    pass

            pending = []

            def flush_pending():
                while pending:
                    rb, ot, h, q0 = pending.pop(0)
                    rc = rcpool.tile([128, QB], F32, tag="rc", name="rc")
                    nc.vector.reciprocal(rc, rb)
                    nc.vector.tensor_mul(ao[:, h, q0:q0 + QB], ot, rc)

            def emit_group(b, h, qb, filler):
                """Attention for one q-head block: transposed scores scheme.

                filler(ns) is called with the ACT-vs-PE time deficit so the
                proj/oproj generator keeps PE busy while ACT computes exps.
                """
                q0 = b * S + qb * QB
                n_kt = (qb + 1) * (QB // 128)
                ot = otpool.tile([128, QB], F32, tag="ot", name="ot")
                sacc = spool.tile([128, QB], F32, tag="S", name="sacc")
                ets = {}
                W = 3

                def emit_sc(kt):
                    c0 = max(0, kt - qb * (QB // 128)) * 128
                    w = QB - c0
                    sc = wpool.tile([128, QB], F32, tag="sc", bufs=3, name="sc")
                    jd = kt - qb * (QB // 128)
                    diag = 0 <= jd < QB // 128
                    nc.tensor.matmul(
                        sc[:, c0:],
                        kt_rot[:, b * S + kt * 128:b * S + (kt + 1) * 128],
                        qt_rot[:, h, q0 + c0:q0 + QB],
                        start=True, stop=not diag, skip_group_check=diag)
                    pe = w * 0.4166
                    if diag:
                        # causal mask folded in on the PE: sc += I.T @ maskT
                        nc.tensor.matmul(sc[:, jd * 128:(jd + 1) * 128],
                                         ident_sb, maskT_sb,
                                         start=False, stop=True,
                                         skip_group_check=True)
                        pe += 128 * 0.4166
                    et = epool.tile([128, QB], BF16, tag="et", name="et")
                    nc.scalar.activation(et[:, c0:], sc[:, c0:], Exp,
                                         bias=0.0, scale=1.0)
                    ets[kt] = (et, c0)
                    return (w * 0.8333 + 230) - pe  # ACT minus PE ns

                deficit = 0.0
                for w in range(min(W, n_kt)):
                    deficit += emit_sc(w)
                filler(max(0.0, deficit))
                for kt in range(n_kt):
                    d = 0.0
                    if kt + W < n_kt:
                        d += emit_sc(kt + W)
                    et, c0 = ets.pop(kt)
                    nc.tensor.matmul(ot[:, c0:], v_sb[:, b * (S // 128) + kt, :],
                                     et[:, c0:], start=(kt == 0),
                                     stop=(kt == n_kt - 1))
                    d -= (QB - c0) * 0.4166
                    if kt == 0:
                        nc.vector.tensor_copy(sacc, et)
                        flush_pending()
                    else:
                        nc.vector.tensor_add(sacc[:, c0:], sacc[:, c0:],
                                             et[:, c0:])
                    filler(max(0.0, d) + 60.0)
                # rowsum via gpsimd partition all-reduce; defer the DVE
                # normalize so the DVE queue never waits on gpsimd
                rb = rbpool.tile([128, QB], F32, tag="rb", name="rb")
                nc.gpsimd.partition_all_reduce(rb, sacc, 128,
                                               bass_isa.ReduceOp.add)
                pending.append((rb, ot, h, q0))

            # ---- startup: fine-grained first DMAs, cheap weights first ----
            nc.vector.memset(warm_sb, 0.0)
            wq_r = wq.ap()
            nc.sync.dma_start(wk_sb[:, 0:8, :], wk.ap()[:, 0:8, :])
            issue_hs(0, 0, slabs=(8, 8, 16))
            nc.sync.dma_start(wk_sb[:, 8:KT, :], wk.ap()[:, 8:KT, :])
            nc.sync.dma_start(wv_sb, wv.ap())
            nc.sync.dma_start(wq_h[0], wq_r[:, 0, :, :])
            nc.sync.dma_start(wq_h[1], wq_r[:, 1, :, :])
            nc.sync.dma_start(wq_h[2], wq_r[:, 2, :, :])
            nc.sync.dma_start(wq_h[3], wq_r[:, 3, :, :])
            nc.sync.dma_start(maskT_sb, maskT.ap())
            nc.sync.dma_start(ident_sb, ident.ap())
            issue_hs(0, 1)
            nc.sync.dma_start(cos_sb, cos_d.ap())
            nc.sync.dma_start(sin_sb, sin_d.ap())
            issue_hs(1, 0)

            # ---- phase 0: chunk-0 projections straight, with warm-up
            # dummies covering the measured DMA-supply stalls ----
            warm0 = {(0, 'k', 0): 16, (0, 'k', 4): 7, (0, 'k', 8): 6,
                     (0, 'k', 16): 10, (0, 'v0', 0): 10, (0, 'q0', 0): 4,
                     (0, 'q1', 0): 3}
            drain(proj_gen(0, warm=warm0))

            # ---- phases 1..3: groups of chunk c-1 + projections of chunk c
            for c in range(1, NCHUNK):
                issue_hs(c, 1)
                if c + 1 < NCHUNK:
                    issue_hs(c + 1, 0)
                if c == NCHUNK - 1:
                    nc.sync.dma_start(wo_sb, wo.ap())
                g = proj_gen(c)
                fill = mk_filler(g)
                pb, pqb = (c - 1) // 2, (c - 1) % 2
                for h in range(HL):
                    emit_group(pb, h, pqb, fill)
                drain(g)

            # ---- phase 4: last chunk's groups + early O-proj tiles ----
            og = oproj_gen(list(range(TT)), split_last=True)
            fill = mk_filler(og)
            pb, pqb = (NCHUNK - 1) // 2, (NCHUNK - 1) % 2
            for h in range(HL):
                emit_group(pb, h, pqb, fill)
            flush_pending()
            # ---- phase 5: rest of the output projection ----
            drain(og)
    nc.compile()
    return nc


def _get_nc():
    if "nc" not in _CACHE:
        _CACHE["nc"] = _build()
    return _CACHE["nc"]


def _prep_inputs(inputs) -> list[dict]:
    bf16 = ml_dtypes.bfloat16
    hs = np.asarray(inputs["hidden_states"], dtype=np.float32).reshape(T, DIM)
    hsT = np.ascontiguousarray(hs.T).astype(bf16)

    fc = np.asarray(inputs["freqs_cos"], dtype=np.float32).reshape(T, HD // 2).T
    fs = np.asarray(inputs["freqs_sin"], dtype=np.float32).reshape(T, HD // 2).T
    cos2 = np.concatenate([fc, fc], axis=0)            # [128, T]
    sin2 = np.concatenate([-fs, fs], axis=0)           # signed half-rotation
    cos_v = np.ascontiguousarray(cos2).astype(bf16)
    sin_v = np.ascontiguousarray(sin2).astype(bf16)

    maskT = np.ascontiguousarray(
        np.asarray(inputs["attention_mask"],
                   dtype=np.float32)[0, 0, :128, :128].T).astype(bf16)
    ident = np.eye(128, dtype=np.float32).astype(bf16)

    perm = np.concatenate([np.arange(0, HD, 2), np.arange(1, HD, 2)])
    Wq = np.asarray(inputs["Wq"], dtype=np.float32) * SCALE  # fold 1/sqrt(d)
    Wk = np.asarray(inputs["Wk"], dtype=np.float32)
    Wv = np.asarray(inputs["Wv"], dtype=np.float32)
    Wo = np.asarray(inputs["Wo"], dtype=np.float32)

    in_maps = []
    for c in range(N_CORES):
        wq_c = np.concatenate(
            [Wq[:, (c * HL + h) * HD:(c * HL + h + 1) * HD][:, perm]
             for h in range(HL)], axis=1)               # [DIM, HL*HD]
        wk_c = Wk[:, c * HD:(c + 1) * HD][:, perm]      # [DIM, HD]
        wv_c = Wv[:, c * HD:(c + 1) * HD]
        wo_c = Wo[c * HL * HD:(c + 1) * HL * HD, :]     # [HL*HD, DIM]
        # repack into [128, ...] SBUF-image layouts (contiguous big descriptors)
        wq_img = wq_c.reshape(KT, 128, HL, HD).transpose(1, 2, 0, 3)
        wk_img = wk_c.reshape(KT, 128, HD).transpose(1, 0, 2)
        wv_img = wv_c.reshape(KT, 128, HD).transpose(1, 0, 2)
        wo_img = wo_c.reshape(HL, 128, DIM).transpose(1, 0, 2)
        in_maps.append({
            "hsT": hsT,
            "wq": np.ascontiguousarray(wq_img).astype(bf16),
            "wk": np.ascontiguousarray(wk_img).astype(bf16),
            "wv": np.ascontiguousarray(wv_img).astype(bf16),
            "wo": np.ascontiguousarray(wo_img).astype(bf16),
            "cos_d": cos_v, "sin_d": sin_v,
            "maskT": maskT, "ident": ident,
        })
    return in_maps


def kernel(**inputs) -> np.ndarray:
    nc = _get_nc()
    in_maps = _prep_inputs(inputs)
    res = bass_utils.run_bass_kernel_spmd(nc, in_maps,
                                          core_ids=list(range(N_CORES)))
    acc = np.zeros((T, DIM), dtype=np.float32)
    for c in range(N_CORES):
        acc += np.asarray(res.results[c]["out"], dtype=np.float32)
    return acc.reshape(B, S, DIM)


# revision 19
# speedup vs baseline: 1.0586x; 1.0586x over previous
"""Trainium2 Bass kernel for MllamaTextSdpaAttention (GQA + RoPE + causal SDPA).

Tensor-parallel over heads across 8 NeuronCores. Core c owns q-heads
[4c, 4c+4) and kv-head c. Each core computes hidden @ Wq/Wk/Wv slices, RoPE,
causal attention for its heads, and its row-slice of the Wo matmul, yielding
a partial [T, DIM] output (bf16) summed on the host in f32.

v2 changes vs the 350.5us baseline:
- Softmax denominators no longer use PE ones-matmuls (36.9k wasted PE
  columns). exp tiles are accumulated elementwise on DVE into S[k%128, q],
  then one gpsimd partition_all_reduce gives the broadcast rowsum; DVE
  reciprocal + multiply normalize. Frees a PSUM bank (work 5 + ot 3).
- Software pipelining: attention groups of chunk c are interleaved with the
  projection matmul chains of chunk c+1 (generator filler), so PE does not
  stall on the ACT exp cadence inside groups. The last chunk's groups
  interleave with early O-projection tiles.
- Weights are repacked host-side into [128, ...] SBUF-image layouts so DMA
  descriptors are >=1KB contiguous (half the per-descriptor latency), and the
  first wq/hs DMAs are sliced fine so the first matmul starts at ~3.5us.
- hs tiles are 256 tokens (3-buf pool), one shared cos/sin table (the 1/sqrt(d)
  scale is folded into Wq host-side), wo stays resident in SBUF.
"""

import numpy as np
import ml_dtypes

import concourse.bacc as bacc
import concourse.bass as bass
import concourse.mybir as mybir
from concourse import bass_isa
from concourse.tile import TileContext
from concourse import bass_utils

BF16 = mybir.dt.bfloat16
F32 = mybir.dt.float32

B, S, DIM = 2, 1024, 4096
T = B * S                     # 2048 tokens, batch-major
N_HEADS, N_KV = 32, 8
HD = 128                      # head dim == partition count
N_CORES = 8
HL = N_HEADS // N_CORES       # 4 local q-heads per core
KT = DIM // 128               # 32 feature tiles
CH = 512                      # chunk (q-block) width
HCH = 256                     # hs half-chunk tile width
NCHUNK = T // CH
QB = 512
TT = T // 128                 # 16 token tiles
SCALE = 1.0 / float(np.sqrt(HD))

_CACHE: dict = {}


def _build():
    nc = bacc.Bacc("TRN2", target_bir_lowering=False, debug=False,
                   enable_asserts=False)

    hsT = nc.dram_tensor("hsT", [DIM, T], BF16, kind="ExternalInput")
    wq = nc.dram_tensor("wq", [128, HL, KT, HD], BF16, kind="ExternalInput")
    wk = nc.dram_tensor("wk", [128, KT, HD], BF16, kind="ExternalInput")
    wv = nc.dram_tensor("wv", [128, KT, HD], BF16, kind="ExternalInput")
    wo = nc.dram_tensor("wo", [128, HL, DIM], BF16, kind="ExternalInput")
    cos_d = nc.dram_tensor("cos_d", [HD, T], BF16, kind="ExternalInput")
    sin_d = nc.dram_tensor("sin_d", [HD, T], BF16, kind="ExternalInput")
    maskT = nc.dram_tensor("maskT", [128, 128], BF16, kind="ExternalInput")
    ident = nc.dram_tensor("ident", [128, 128], BF16, kind="ExternalInput")
    out = nc.dram_tensor("out", [T, DIM], BF16, kind="ExternalOutput")

    Exp = mybir.ActivationFunctionType.Exp

    with TileContext(nc) as tc:
        with tc.tile_pool(name="consts", bufs=1) as cpool, \
             tc.tile_pool(name="hs", bufs=3) as hpool, \
             tc.tile_pool(name="rope_tmp", bufs=2) as rpool, \
             tc.tile_pool(name="work_ps", bufs=5, space=bass.MemorySpace.PSUM) as wpool, \
             tc.tile_pool(name="ot_ps", bufs=3, space=bass.MemorySpace.PSUM) as otpool, \
             tc.tile_pool(name="et", bufs=6) as epool, \
             tc.tile_pool(name="ssum", bufs=3) as spool, \
             tc.tile_pool(name="rsbc", bufs=2) as rbpool, \
             tc.tile_pool(name="recip", bufs=2) as rcpool, \
             tc.tile_pool(name="out_sb", bufs=6) as xsbpool:

            wq_h = [cpool.tile([128, KT, HD], BF16, tag=f"wq{m}", name=f"wq{m}")
                    for m in range(HL)]
            wk_sb = cpool.tile([128, KT, HD], BF16, tag="wk")
            wv_sb = cpool.tile([128, KT, HD], BF16, tag="wv")
            cos_sb = cpool.tile([128, T], BF16, tag="cos")
            sin_sb = cpool.tile([128, T], BF16, tag="sin")
            maskT_sb = cpool.tile([128, 128], BF16, tag="maskT")
            ident_sb = cpool.tile([128, 128], BF16, tag="ident")
            qt_rot = cpool.tile([128, HL, T], BF16, tag="qt")
            kt_rot = cpool.tile([128, T], BF16, tag="kt")
            v_sb = cpool.tile([128, TT, HD], BF16, tag="v")
            ao = cpool.tile([128, HL, T], BF16, tag="ao")
            wo_sb = cpool.tile([128, HL, DIM], BF16, tag="wo")

            hsT_r = hsT.ap().rearrange("(kt p) t -> p kt t", p=128)
            hs_tiles: dict = {}

            def issue_hs(c, half, slabs=(16, 16)):
                """DMA one [128, KT, HCH] hs half-chunk, split into kt-slabs."""
                t0 = c * CH + half * HCH
                tile = hpool.tile([128, KT, HCH], BF16, tag="hs",
                                  name=f"hs{c}_{half}")
                hs_tiles[(c, half)] = tile
                k0 = 0
                for w in slabs:
                    nc.sync.dma_start(tile[:, k0:k0 + w, :],
                                      hsT_r[:, k0:k0 + w, t0:t0 + HCH])
                    k0 += w
                assert k0 == KT

            def rope(ps, out_ap, t0):
                """out = ps*cos + halfswap(ps)*sin (signs baked into sin)."""
                c_ap = cos_sb[:, t0:t0 + HCH]
                s_ap = sin_sb[:, t0:t0 + HCH]
                t1 = rpool.tile([128, HCH], F32, tag="r1", name="t1")
                t2 = rpool.tile([128, HCH], F32, tag="r2", name="t2")
                nc.vector.tensor_mul(t1, ps, c_ap)
                nc.vector.tensor_mul(t2[0:64, :], ps[64:128, :], s_ap[0:64, :])
                nc.vector.tensor_mul(t2[64:128, :], ps[0:64, :], s_ap[64:128, :])
                nc.vector.tensor_add(out_ap, t1, t2)

            warm_sb = cpool.tile([128, 256], BF16, tag="warm_sb")
            warm_ps = otpool.tile([128, 512], F32, tag="ot", name="warm_ps")

            def dummy(n):
                """Warm-up matmuls on a memset tile: keep PE busy (and its
                p-state ramped) through DMA-paced stretches. Output is
                discarded."""
                for _ in range(n):
                    nc.tensor.matmul(warm_ps[:, 0:256], warm_sb[:, 0:128],
                                     warm_sb, start=True, stop=True)

            def proj_gen(c, warm=None):
                """Projection chains for chunk c (K -> V -> Q per half so the
                cheap weights unlock PE first); yields PE-ns after each
                matmul. warm: dict (half, chain, kt) -> dummy count emitted
                before that matmul (phase-0 DMA pacing)."""
                def pad(half, chain, kt):
                    if warm:
                        dummy(warm.get((half, chain, kt), 0))

                for half in range(2):
                    t0 = c * CH + half * HCH
                    hs = hs_tiles[(c, half)]
                    ps = wpool.tile([128, HCH], F32, tag="work", name="ps_k")
                    for kt in range(KT):
                        pad(half, 'k', kt)
                        nc.tensor.matmul(ps, wk_sb[:, kt, :], hs[:, kt, :],
                                         start=(kt == 0), stop=(kt == KT - 1))
                        yield HCH * 0.4166
                    rope(ps, kt_rot[:, t0:t0 + HCH], t0)
                    for vi in range(HCH // 128):
                        tt = t0 // 128 + vi
                        ps = wpool.tile([128, HD], F32, tag="work", name="ps_v")
                        for kt in range(KT):
                            pad(half, f'v{vi}', kt)
                            nc.tensor.matmul(ps,
                                             hs[:, kt, vi * 128:(vi + 1) * 128],
                                             wv_sb[:, kt, :],
                                             start=(kt == 0), stop=(kt == KT - 1))
                            yield HD * 0.4166
                        nc.scalar.copy(v_sb[:, tt, :], ps)
                    for m in range(HL):
                        ps = wpool.tile([128, HCH], F32, tag="work", name="ps_q")
                        for kt in range(KT):
                            pad(half, f'q{m}', kt)
                            nc.tensor.matmul(ps, wq_h[m][:, kt, :], hs[:, kt, :],
                                             start=(kt == 0), stop=(kt == KT - 1))
                            yield HCH * 0.4166
                        rope(ps, qt_rot[:, m, t0:t0 + HCH], t0)

            def oproj_gen(tts, split_last=False):
                """Output projection tiles; yields PE-ns after each matmul."""
                last = (tts[-1], DIM // 512 - 1)
                for tt in tts:
                    for ni in range(DIM // 512):
                        ps = wpool.tile([128, 512], F32, tag="work", name="ps_o")
                        for kh in range(HL):
                            nc.tensor.matmul(ps, ao[:, kh, tt * 128:(tt + 1) * 128],
                                             wo_sb[:, kh, ni * 512:(ni + 1) * 512],
                                             start=(kh == 0), stop=(kh == HL - 1))
                            yield 512 * 0.4166
                        osb = xsbpool.tile([128, 512], BF16, tag="osb", name="osb")
                        if split_last and (tt, ni) == last:
                            # two half copies/DMAs to shorten the final drain
                            nc.scalar.copy(osb[:, 0:256], ps[:, 0:256])
                            nc.sync.dma_start(
                                out.ap()[tt * 128:(tt + 1) * 128,
                                         ni * 512:ni * 512 + 256], osb[:, 0:256])
                            nc.vector.tensor_copy(osb[:, 256:], ps[:, 256:])
                            nc.sync.dma_start(
                                out.ap()[tt * 128:(tt + 1) * 128,
                                         ni * 512 + 256:(ni + 1) * 512],
                                osb[:, 256:])
                            continue
                        if (tt * 8 + ni) % 2 == 0:
                            nc.scalar.copy(osb, ps)
                        else:
                            nc.vector.tensor_copy(osb, ps)
                        nc.sync.dma_start(
                            out.ap()[tt * 128:(tt + 1) * 128,
                                     ni * 512:(ni + 1) * 512], osb)

            def mk_filler(gen):
                state = {'bank': 0.0, 'done': False}

                def filler(ns):
                    state['bank'] -= ns
                    while state['bank'] < 0 and not state['done']:
                        got = next(gen, None)
                        if got is None:
                            state['done'] = True
                            return
                        state['bank'] += got
                return filler

            def drain(gen):
                for _ in gen:
                    pass

            pending = []

            def flush_pending():
                while pending:
                    rb, ot, h, q0 = pending.pop(0)
                    rc = rcpool.tile([128, QB], F32, tag="rc", name="rc")
                    nc.vector.reciprocal(rc, rb)
                    nc.vector.tensor_mul(ao[:, h, q0:q0 + QB], ot, rc)

            def emit_group(b, h, qb, filler):
                """Attention for one q-head block: transposed scores scheme.

                filler(ns) is called with the ACT-vs-PE time deficit so the
                proj/oproj generator keeps PE busy while ACT computes exps.
                """
                q0 = b * S + qb * QB
                n_kt = (qb + 1) * (QB // 128)
                ot = otpool.tile([128, QB], F32, tag="ot", name="ot")
                sacc = spool.tile([128, QB], F32, tag="S", name="sacc")
                ets = {}
                W = 3

                def emit_sc(kt):
                    c0 = max(0, kt - qb * (QB // 128)) * 128
                    w = QB - c0
                    sc = wpool.tile([128, QB], F32, tag="work", name="sc")
                    jd = kt - qb * (QB // 128)
                    diag = 0 <= jd < QB // 128
                    nc.tensor.matmul(
                        sc[:, c0:],
                        kt_rot[:, b * S + kt * 128:b * S + (kt + 1) * 128],
                        qt_rot[:, h, q0 + c0:q0 + QB],
                        start=True, stop=not diag, skip_group_check=diag)
                    pe = w * 0.4166
                    if diag:
                        # causal mask folded in on the PE: sc += I.T @ maskT
                        nc.tensor.matmul(sc[:, jd * 128:(jd + 1) * 128],
                                         ident_sb, maskT_sb,
                                         start=False, stop=True,
                                         skip_group_check=True)
                        pe += 128 * 0.4166
                    et = epool.tile([128, QB], BF16, tag="et", name="et")
                    nc.scalar.activation(et[:, c0:], sc[:, c0:], Exp,
                                         bias=0.0, scale=1.0)
                    ets[kt] = (et, c0)
                    return (w * 0.8333 + 230) - pe  # ACT minus PE ns

                deficit = 0.0
                for w in range(min(W, n_kt)):
                    deficit += emit_sc(w)
                filler(max(0.0, deficit))
                for kt in range(n_kt):
                    d = 0.0
                    if kt + W < n_kt:
                        d += emit_sc(kt + W)
                    et, c0 = ets.pop(kt)
                    nc.tensor.matmul(ot[:, c0:], v_sb[:, b * (S // 128) + kt, :],
                                     et[:, c0:], start=(kt == 0),
                                     stop=(kt == n_kt - 1))
                    d -= (QB - c0) * 0.4166
                    if kt == 0:
                        nc.vector.tensor_copy(sacc, et)
                        flush_pending()
                    else:
                        nc.vector.tensor_add(sacc[:, c0:], sacc[:, c0:],
                                             et[:, c0:])
                    filler(max(0.0, d) + 60.0)
                # rowsum via gpsimd partition all-reduce; defer the DVE
                # normalize so the DVE queue never waits on gpsimd
                rb = rbpool.tile([128, QB], F32, tag="rb", name="rb")
                nc.gpsimd.partition_all_reduce(rb, sacc, 128,
                                               bass_isa.ReduceOp.add)
                pending.append((rb, ot, h, q0))

            # ---- startup: fine-grained first DMAs, cheap weights first ----
            nc.vector.memset(warm_sb, 0.0)
            wq_r = wq.ap()
            nc.sync.dma_start(wk_sb[:, 0:8, :], wk.ap()[:, 0:8, :])
            issue_hs(0, 0, slabs=(8, 8, 16))
            nc.sync.dma_start(wk_sb[:, 8:KT, :], wk.ap()[:, 8:KT, :])
            nc.sync.dma_start(wv_sb, wv.ap())
            nc.sync.dma_start(wq_h[0], wq_r[:, 0, :, :])
            nc.sync.dma_start(wq_h[1], wq_r[:, 1, :, :])
            nc.sync.dma_start(wq_h[2], wq_r[:, 2, :, :])
            nc.sync.dma_start(wq_h[3], wq_r[:, 3, :, :])
            nc.sync.dma_start(maskT_sb, maskT.ap())
            nc.sync.dma_start(ident_sb, ident.ap())
            issue_hs(0, 1)
            nc.sync.dma_start(cos_sb, cos_d.ap())
            nc.sync.dma_start(sin_sb, sin_d.ap())
            issue_hs(1, 0)

            # ---- phase 0: chunk-0 projections straight, with warm-up
            # dummies covering the measured DMA-supply stalls ----
            warm0 = {(0, 'k', 0): 16, (0, 'k', 4): 7, (0, 'k', 8): 6,
                     (0, 'k', 16): 10, (0, 'v0', 0): 10, (0, 'q0', 0): 4,
                     (0, 'q1', 0): 3}
            drain(proj_gen(0, warm=warm0))

            # ---- phases 1..3: groups of chunk c-1 + projections of chunk c
            for c in range(1, NCHUNK):
                issue_hs(c, 1)
                if c + 1 < NCHUNK:
                    issue_hs(c + 1, 0)
                if c == NCHUNK - 1:
                    nc.sync.dma_start(wo_sb, wo.ap())
                g = proj_gen(c)
                fill = mk_filler(g)
                pb, pqb = (c - 1) // 2, (c - 1) % 2
                for h in range(HL):
                    emit_group(pb, h, pqb, fill)
                drain(g)

            # ---- phase 4: last chunk's groups + early O-proj tiles ----
            og = oproj_gen(list(range(TT)), split_last=True)
            fill = mk_filler(og)
            pb, pqb = (NCHUNK - 1) // 2, (NCHUNK - 1) % 2
            for h in range(HL):
                emit_group(pb, h, pqb, fill)
            flush_pending()
            # ---- phase 5: rest of the output projection ----
            drain(og)
    nc.compile()
    return nc


def _get_nc():
    if "nc" not in _CACHE:
        _CACHE["nc"] = _build()
    return _CACHE["nc"]


def _prep_inputs(inputs) -> list[dict]:
    bf16 = ml_dtypes.bfloat16
    hs = np.asarray(inputs["hidden_states"], dtype=np.float32).reshape(T, DIM)
    hsT = np.ascontiguousarray(hs.T).astype(bf16)

    fc = np.asarray(inputs["freqs_cos"], dtype=np.float32).reshape(T, HD // 2).T
    fs = np.asarray(inputs["freqs_sin"], dtype=np.float32).reshape(T, HD // 2).T
    cos2 = np.concatenate([fc, fc], axis=0)            # [128, T]
    sin2 = np.concatenate([-fs, fs], axis=0)           # signed half-rotation
    cos_v = np.ascontiguousarray(cos2).astype(bf16)
    sin_v = np.ascontiguousarray(sin2).astype(bf16)

    maskT = np.ascontiguousarray(
        np.asarray(inputs["attention_mask"],
                   dtype=np.float32)[0, 0, :128, :128].T).astype(bf16)
    ident = np.eye(128, dtype=np.float32).astype(bf16)

    perm = np.concatenate([np.arange(0, HD, 2), np.arange(1, HD, 2)])
    Wq = np.asarray(inputs["Wq"], dtype=np.float32) * SCALE  # fold 1/sqrt(d)
    Wk = np.asarray(inputs["Wk"], dtype=np.float32)
    Wv = np.asarray(inputs["Wv"], dtype=np.float32)
    Wo = np.asarray(inputs["Wo"], dtype=np.float32)

    in_maps = []
    for c in range(N_CORES):
        wq_c = np.concatenate(
            [Wq[:, (c * HL + h) * HD:(c * HL + h + 1) * HD][:, perm]
             for h in range(HL)], axis=1)               # [DIM, HL*HD]
        wk_c = Wk[:, c * HD:(c + 1) * HD][:, perm]      # [DIM, HD]
        wv_c = Wv[:, c * HD:(c + 1) * HD]
        wo_c = Wo[c * HL * HD:(c + 1) * HL * HD, :]     # [HL*HD, DIM]
        # repack into [128, ...] SBUF-image layouts (contiguous big descriptors)
        wq_img = wq_c.reshape(KT, 128, HL, HD).transpose(1, 2, 0, 3)
        wk_img = wk_c.reshape(KT, 128, HD).transpose(1, 0, 2)
        wv_img = wv_c.reshape(KT, 128, HD).transpose(1, 0, 2)
        wo_img = wo_c.reshape(HL, 128, DIM).transpose(1, 0, 2)
        in_maps.append({
            "hsT": hsT,
            "wq": np.ascontiguousarray(wq_img).astype(bf16),
            "wk": np.ascontiguousarray(wk_img).astype(bf16),
            "wv": np.ascontiguousarray(wv_img).astype(bf16),
            "wo": np.ascontiguousarray(wo_img).astype(bf16),
            "cos_d": cos_v, "sin_d": sin_v,
            "maskT": maskT, "ident": ident,
        })
    return in_maps


def kernel(**inputs) -> np.ndarray:
    nc = _get_nc()
    in_maps = _prep_inputs(inputs)
    res = bass_utils.run_bass_kernel_spmd(nc, in_maps,
                                          core_ids=list(range(N_CORES)))
    acc = np.zeros((T, DIM), dtype=np.float32)
    for c in range(N_CORES):
        acc += np.asarray(res.results[c]["out"], dtype=np.float32)
    return acc.reshape(B, S, DIM)


# revision 20
# speedup vs baseline: 1.0752x; 1.0157x over previous
"""Trainium2 Bass kernel for MllamaTextSdpaAttention (GQA + RoPE + causal SDPA).

Tensor-parallel over heads across 8 NeuronCores. Core c owns q-heads
[4c, 4c+4) and kv-head c. Each core computes hidden @ Wq/Wk/Wv slices, RoPE,
causal attention for its heads, and its row-slice of the Wo matmul, yielding
a partial [T, DIM] output (bf16) summed on the host in f32.

v2 changes vs the 350.5us baseline:
- Softmax denominators no longer use PE ones-matmuls (36.9k wasted PE
  columns). exp tiles are accumulated elementwise on DVE into S[k%128, q],
  then one gpsimd partition_all_reduce gives the broadcast rowsum; DVE
  reciprocal + multiply normalize. Frees a PSUM bank (work 5 + ot 3).
- Software pipelining: attention groups of chunk c are interleaved with the
  projection matmul chains of chunk c+1 (generator filler), so PE does not
  stall on the ACT exp cadence inside groups. The last chunk's groups
  interleave with early O-projection tiles.
- Weights are repacked host-side into [128, ...] SBUF-image layouts so DMA
  descriptors are >=1KB contiguous (half the per-descriptor latency), and the
  first wq/hs DMAs are sliced fine so the first matmul starts at ~3.5us.
- hs tiles are 256 tokens (3-buf pool), one shared cos/sin table (the 1/sqrt(d)
  scale is folded into Wq host-side), wo stays resident in SBUF.
"""

import numpy as np
import ml_dtypes

import concourse.bacc as bacc
import concourse.bass as bass
import concourse.mybir as mybir
from concourse import bass_isa
from concourse.tile import TileContext
from concourse import bass_utils

BF16 = mybir.dt.bfloat16
F32 = mybir.dt.float32

B, S, DIM = 2, 1024, 4096
T = B * S                     # 2048 tokens, batch-major
N_HEADS, N_KV = 32, 8
HD = 128                      # head dim == partition count
N_CORES = 8
HL = N_HEADS // N_CORES       # 4 local q-heads per core
KT = DIM // 128               # 32 feature tiles
CH = 512                      # chunk (q-block) width
HCH = 256                     # hs half-chunk tile width
NCHUNK = T // CH
QB = 512
TT = T // 128                 # 16 token tiles
SCALE = 1.0 / float(np.sqrt(HD))

_CACHE: dict = {}


def _build():
    nc = bacc.Bacc("TRN2", target_bir_lowering=False, debug=False,
                   enable_asserts=False)

    hsT = nc.dram_tensor("hsT", [DIM, T], BF16, kind="ExternalInput")
    wq = nc.dram_tensor("wq", [128, HL, KT, HD], BF16, kind="ExternalInput")
    wk = nc.dram_tensor("wk", [128, KT, HD], BF16, kind="ExternalInput")
    wv = nc.dram_tensor("wv", [128, KT, HD], BF16, kind="ExternalInput")
    wo = nc.dram_tensor("wo", [128, HL, DIM], BF16, kind="ExternalInput")
    cos_d = nc.dram_tensor("cos_d", [HD, T], BF16, kind="ExternalInput")
    sin_d = nc.dram_tensor("sin_d", [HD, T], BF16, kind="ExternalInput")
    maskT = nc.dram_tensor("maskT", [128, 128], BF16, kind="ExternalInput")
    ident = nc.dram_tensor("ident", [128, 128], BF16, kind="ExternalInput")
    out = nc.dram_tensor("out", [T, DIM], BF16, kind="ExternalOutput")

    Exp = mybir.ActivationFunctionType.Exp

    with TileContext(nc) as tc:
        with tc.tile_pool(name="consts", bufs=1) as cpool, \
             tc.tile_pool(name="hs", bufs=3) as hpool, \
             tc.tile_pool(name="rope_tmp", bufs=2) as rpool, \
             tc.tile_pool(name="work_ps", bufs=5, space=bass.MemorySpace.PSUM) as wpool, \
             tc.tile_pool(name="ot_ps", bufs=3, space=bass.MemorySpace.PSUM) as otpool, \
             tc.tile_pool(name="et", bufs=6) as epool, \
             tc.tile_pool(name="ssum", bufs=3) as spool, \
             tc.tile_pool(name="rsbc", bufs=2) as rbpool, \
             tc.tile_pool(name="recip", bufs=2) as rcpool, \
             tc.tile_pool(name="out_sb", bufs=6) as xsbpool:

            wq_h = [cpool.tile([128, KT, HD], BF16, tag=f"wq{m}", name=f"wq{m}")
                    for m in range(HL)]
            wk_sb = cpool.tile([128, KT, HD], BF16, tag="wk")
            wv_sb = cpool.tile([128, KT, HD], BF16, tag="wv")
            cos_sb = cpool.tile([128, T], BF16, tag="cos")
            sin_sb = cpool.tile([128, T], BF16, tag="sin")
            maskT_sb = cpool.tile([128, 128], BF16, tag="maskT")
            ident_sb = cpool.tile([128, 128], BF16, tag="ident")
            qt_rot = cpool.tile([128, HL, T], BF16, tag="qt")
            kt_rot = cpool.tile([128, T], BF16, tag="kt")
            v_sb = cpool.tile([128, TT, HD], BF16, tag="v")
            ao = cpool.tile([128, HL, T], BF16, tag="ao")
            wo_sb = cpool.tile([128, HL, DIM], BF16, tag="wo")

            hsT_r = hsT.ap().rearrange("(kt p) t -> p kt t", p=128)
            hs_tiles: dict = {}

            def issue_hs(c, half, slabs=(16, 16)):
                """DMA one [128, KT, HCH] hs half-chunk, split into kt-slabs."""
                t0 = c * CH + half * HCH
                tile = hpool.tile([128, KT, HCH], BF16, tag="hs",
                                  name=f"hs{c}_{half}")
                hs_tiles[(c, half)] = tile
                k0 = 0
                for w in slabs:
                    nc.sync.dma_start(tile[:, k0:k0 + w, :],
                                      hsT_r[:, k0:k0 + w, t0:t0 + HCH])
                    k0 += w
                assert k0 == KT

            def rope(ps, out_ap, t0):
                """out = ps*cos + halfswap(ps)*sin (signs baked into sin)."""
                c_ap = cos_sb[:, t0:t0 + HCH]
                s_ap = sin_sb[:, t0:t0 + HCH]
                t1 = rpool.tile([128, HCH], F32, tag="r1", name="t1")
                t2 = rpool.tile([128, HCH], F32, tag="r2", name="t2")
                nc.vector.tensor_mul(t1, ps, c_ap)
                nc.vector.tensor_mul(t2[0:64, :], ps[64:128, :], s_ap[0:64, :])
                nc.vector.tensor_mul(t2[64:128, :], ps[0:64, :], s_ap[64:128, :])
                nc.vector.tensor_add(out_ap, t1, t2)

            warm_sb = cpool.tile([128, 256], BF16, tag="warm_sb")
            warm_ps = otpool.tile([128, 512], F32, tag="ot", name="warm_ps")

            def dummy(n):
                """Warm-up matmuls on a memset tile: keep PE busy (and its
                p-state ramped) through DMA-paced stretches. Output is
                discarded."""
                for _ in range(n):
                    nc.tensor.matmul(warm_ps[:, 0:256], warm_sb[:, 0:128],
                                     warm_sb, start=True, stop=True)

            def proj_gen(c, warm=None):
                """Projection chains for chunk c (K -> V -> Q per half so the
                cheap weights unlock PE first); yields PE-ns after each
                matmul. warm: dict (half, chain, kt) -> dummy count emitted
                before that matmul (phase-0 DMA pacing)."""
                def pad(half, chain, kt):
                    if warm:
                        dummy(warm.get((half, chain, kt), 0))

                for half in range(2):
                    t0 = c * CH + half * HCH
                    hs = hs_tiles[(c, half)]
                    ps = wpool.tile([128, HCH], F32, tag="work", name="ps_k")
                    for kt in range(KT):
                        pad(half, 'k', kt)
                        nc.tensor.matmul(ps, wk_sb[:, kt, :], hs[:, kt, :],
                                         start=(kt == 0), stop=(kt == KT - 1))
                        yield HCH * 0.4166
                    rope(ps, kt_rot[:, t0:t0 + HCH], t0)
                    for vi in range(HCH // 128):
                        tt = t0 // 128 + vi
                        ps = wpool.tile([128, HD], F32, tag="work", name="ps_v")
                        for kt in range(KT):
                            pad(half, f'v{vi}', kt)
                            nc.tensor.matmul(ps,
                                             hs[:, kt, vi * 128:(vi + 1) * 128],
                                             wv_sb[:, kt, :],
                                             start=(kt == 0), stop=(kt == KT - 1))
                            yield HD * 0.4166
                        nc.scalar.copy(v_sb[:, tt, :], ps)
                    for m in range(HL):
                        ps = wpool.tile([128, HCH], F32, tag="work", name="ps_q")
                        for kt in range(KT):
                            pad(half, f'q{m}', kt)
                            nc.tensor.matmul(ps, wq_h[m][:, kt, :], hs[:, kt, :],
                                             start=(kt == 0), stop=(kt == KT - 1))
                            yield HCH * 0.4166
                        rope(ps, qt_rot[:, m, t0:t0 + HCH], t0)

            def oproj_gen(tts, split_last=False):
                """Output projection tiles; yields PE-ns after each matmul."""
                last = (tts[-1], DIM // 512 - 1)
                for tt in tts:
                    for ni in range(DIM // 512):
                        ps = wpool.tile([128, 512], F32, tag="work", name="ps_o")
                        for kh in range(HL):
                            nc.tensor.matmul(ps, ao[:, kh, tt * 128:(tt + 1) * 128],
                                             wo_sb[:, kh, ni * 512:(ni + 1) * 512],
                                             start=(kh == 0), stop=(kh == HL - 1))
                            yield 512 * 0.4166
                        osb = xsbpool.tile([128, 512], BF16, tag="osb", name="osb")
                        if split_last and (tt, ni) == last:
                            # two half copies/DMAs to shorten the final drain
                            nc.scalar.copy(osb[:, 0:256], ps[:, 0:256])
                            nc.sync.dma_start(
                                out.ap()[tt * 128:(tt + 1) * 128,
                                         ni * 512:ni * 512 + 256], osb[:, 0:256])
                            nc.vector.tensor_copy(osb[:, 256:], ps[:, 256:])
                            nc.sync.dma_start(
                                out.ap()[tt * 128:(tt + 1) * 128,
                                         ni * 512 + 256:(ni + 1) * 512],
                                osb[:, 256:])
                            continue
                        if (tt * 8 + ni) % 2 == 0:
                            nc.scalar.copy(osb, ps)
                        else:
                            nc.vector.tensor_copy(osb, ps)
                        nc.sync.dma_start(
                            out.ap()[tt * 128:(tt + 1) * 128,
                                     ni * 512:(ni + 1) * 512], osb)

            def mk_filler(gen):
                state = {'bank': 0.0, 'done': False}

                def filler(ns):
                    state['bank'] -= ns
                    while state['bank'] < 0 and not state['done']:
                        got = next(gen, None)
                        if got is None:
                            state['done'] = True
                            return
                        state['bank'] += got
                return filler

            def drain(gen):
                for _ in gen:
                    pass

            pending = []

            def flush_pending():
                while pending:
                    rb, ot, h, q0 = pending.pop(0)
                    rc = rcpool.tile([128, QB], F32, tag="rc", name="rc")
                    nc.vector.reciprocal(rc, rb)
                    nc.vector.tensor_mul(ao[:, h, q0:q0 + QB], ot, rc)

            def emit_group(b, h, qb, filler):
                """Attention for one q-head block: transposed scores scheme.

                filler(ns) is called with the ACT-vs-PE time deficit so the
                proj/oproj generator keeps PE busy while ACT computes exps.
                """
                q0 = b * S + qb * QB
                n_kt = (qb + 1) * (QB // 128)
                ot = otpool.tile([128, QB], F32, tag="ot", name="ot")
                sacc = spool.tile([128, QB], F32, tag="S", name="sacc")
                ets = {}
                W = 3

                def emit_sc(kt):
                    c0 = max(0, kt - qb * (QB // 128)) * 128
                    w = QB - c0
                    sc = wpool.tile([128, QB], F32, tag="work", name="sc")
                    jd = kt - qb * (QB // 128)
                    diag = 0 <= jd < QB // 128
                    nc.tensor.matmul(
                        sc[:, c0:],
                        kt_rot[:, b * S + kt * 128:b * S + (kt + 1) * 128],
                        qt_rot[:, h, q0 + c0:q0 + QB],
                        start=True, stop=not diag, skip_group_check=diag)
                    pe = w * 0.4166
                    if diag:
                        # causal mask folded in on the PE: sc += I.T @ maskT
                        nc.tensor.matmul(sc[:, jd * 128:(jd + 1) * 128],
                                         ident_sb, maskT_sb,
                                         start=False, stop=True,
                                         skip_group_check=True)
                        pe += 128 * 0.4166
                    et = epool.tile([128, QB], BF16, tag="et", name="et")
                    nc.scalar.activation(et[:, c0:], sc[:, c0:], Exp,
                                         bias=0.0, scale=1.0)
                    ets[kt] = (et, c0)
                    return (w * 0.8333 + 230) - pe  # ACT minus PE ns

                deficit = 0.0
                for w in range(min(W, n_kt)):
                    deficit += emit_sc(w)
                filler(max(0.0, deficit))
                for kt in range(n_kt):
                    d = 0.0
                    if kt + W < n_kt:
                        d += emit_sc(kt + W)
                    et, c0 = ets.pop(kt)
                    nc.tensor.matmul(ot[:, c0:], v_sb[:, b * (S // 128) + kt, :],
                                     et[:, c0:], start=(kt == 0),
                                     stop=(kt == n_kt - 1))
                    d -= (QB - c0) * 0.4166
                    if kt == 0:
                        nc.vector.tensor_copy(sacc, et)
                        flush_pending()
                    else:
                        nc.vector.tensor_add(sacc[:, c0:], sacc[:, c0:],
                                             et[:, c0:])
                    filler(max(0.0, d) + 60.0)
                # rowsum via gpsimd partition all-reduce; defer the DVE
                # normalize so the DVE queue never waits on gpsimd
                rb = rbpool.tile([128, QB], F32, tag="rb", name="rb")
                nc.gpsimd.partition_all_reduce(rb, sacc, 128,
                                               bass_isa.ReduceOp.add)
                pending.append((rb, ot, h, q0))

            # ---- startup: fine-grained first DMAs, cheap weights first ----
            nc.vector.memset(warm_sb, 0.0)
            wq_r = wq.ap()
            nc.sync.dma_start(wk_sb[:, 0:8, :], wk.ap()[:, 0:8, :])
            issue_hs(0, 0, slabs=(8, 8, 16))
            nc.sync.dma_start(wk_sb[:, 8:KT, :], wk.ap()[:, 8:KT, :])
            nc.sync.dma_start(wv_sb, wv.ap())
            nc.sync.dma_start(wq_h[0], wq_r[:, 0, :, :])
            nc.sync.dma_start(wq_h[1], wq_r[:, 1, :, :])
            nc.sync.dma_start(wq_h[2], wq_r[:, 2, :, :])
            nc.sync.dma_start(wq_h[3], wq_r[:, 3, :, :])
            nc.sync.dma_start(maskT_sb, maskT.ap())
            nc.sync.dma_start(ident_sb, ident.ap())
            issue_hs(0, 1)
            nc.sync.dma_start(cos_sb, cos_d.ap())
            nc.sync.dma_start(sin_sb, sin_d.ap())
            issue_hs(1, 0)

            # ---- phase 0: chunk-0 projections straight, with warm-up
            # dummies covering the measured DMA-supply stalls ----
            warm0 = {(0, 'k', 0): 16, (0, 'k', 4): 7, (0, 'k', 8): 6,
                     (0, 'k', 16): 10, (0, 'v0', 0): 10, (0, 'q0', 0): 4,
                     (0, 'q1', 0): 3}
            drain(proj_gen(0))

            # ---- phases 1..3: groups of chunk c-1 + projections of chunk c
            for c in range(1, NCHUNK):
                issue_hs(c, 1)
                if c + 1 < NCHUNK:
                    issue_hs(c + 1, 0)
                if c == NCHUNK - 1:
                    nc.sync.dma_start(wo_sb, wo.ap())
                g = proj_gen(c)
                fill = mk_filler(g)
                pb, pqb = (c - 1) // 2, (c - 1) % 2
                for h in range(HL):
                    emit_group(pb, h, pqb, fill)
                drain(g)

            # ---- phase 4: last chunk's groups + early O-proj tiles ----
            og = oproj_gen(list(range(TT)), split_last=True)
            fill = mk_filler(og)
            pb, pqb = (NCHUNK - 1) // 2, (NCHUNK - 1) % 2
            for h in range(HL):
                emit_group(pb, h, pqb, fill)
            flush_pending()
            # ---- phase 5: rest of the output projection ----
            drain(og)
    nc.compile()
    return nc


def _get_nc():
    if "nc" not in _CACHE:
        _CACHE["nc"] = _build()
    return _CACHE["nc"]


def _prep_inputs(inputs) -> list[dict]:
    bf16 = ml_dtypes.bfloat16
    hs = np.asarray(inputs["hidden_states"], dtype=np.float32).reshape(T, DIM)
    hsT = np.ascontiguousarray(hs.T).astype(bf16)

    fc = np.asarray(inputs["freqs_cos"], dtype=np.float32).reshape(T, HD // 2).T
    fs = np.asarray(inputs["freqs_sin"], dtype=np.float32).reshape(T, HD // 2).T
    cos2 = np.concatenate([fc, fc], axis=0)            # [128, T]
    sin2 = np.concatenate([-fs, fs], axis=0)           # signed half-rotation
    cos_v = np.ascontiguousarray(cos2).astype(bf16)
    sin_v = np.ascontiguousarray(sin2).astype(bf16)

    maskT = np.ascontiguousarray(
        np.asarray(inputs["attention_mask"],
                   dtype=np.float32)[0, 0, :128, :128].T).astype(bf16)
    ident = np.eye(128, dtype=np.float32).astype(bf16)

    perm = np.concatenate([np.arange(0, HD, 2), np.arange(1, HD, 2)])
    Wq = np.asarray(inputs["Wq"], dtype=np.float32) * SCALE  # fold 1/sqrt(d)
    Wk = np.asarray(inputs["Wk"], dtype=np.float32)
    Wv = np.asarray(inputs["Wv"], dtype=np.float32)
    Wo = np.asarray(inputs["Wo"], dtype=np.float32)

    in_maps = []
    for c in range(N_CORES):
        wq_c = np.concatenate(
            [Wq[:, (c * HL + h) * HD:(c * HL + h + 1) * HD][:, perm]
             for h in range(HL)], axis=1)               # [DIM, HL*HD]
        wk_c = Wk[:, c * HD:(c + 1) * HD][:, perm]      # [DIM, HD]
        wv_c = Wv[:, c * HD:(c + 1) * HD]
        wo_c = Wo[c * HL * HD:(c + 1) * HL * HD, :]     # [HL*HD, DIM]
        # repack into [128, ...] SBUF-image layouts (contiguous big descriptors)
        wq_img = wq_c.reshape(KT, 128, HL, HD).transpose(1, 2, 0, 3)
        wk_img = wk_c.reshape(KT, 128, HD).transpose(1, 0, 2)
        wv_img = wv_c.reshape(KT, 128, HD).transpose(1, 0, 2)
        wo_img = wo_c.reshape(HL, 128, DIM).transpose(1, 0, 2)
        in_maps.append({
            "hsT": hsT,
            "wq": np.ascontiguousarray(wq_img).astype(bf16),
            "wk": np.ascontiguousarray(wk_img).astype(bf16),
            "wv": np.ascontiguousarray(wv_img).astype(bf16),
            "wo": np.ascontiguousarray(wo_img).astype(bf16),
            "cos_d": cos_v, "sin_d": sin_v,
            "maskT": maskT, "ident": ident,
        })
    return in_maps


def kernel(**inputs) -> np.ndarray:
    nc = _get_nc()
    in_maps = _prep_inputs(inputs)
    res = bass_utils.run_bass_kernel_spmd(nc, in_maps,
                                          core_ids=list(range(N_CORES)))
    acc = np.zeros((T, DIM), dtype=np.float32)
    for c in range(N_CORES):
        acc += np.asarray(res.results[c]["out"], dtype=np.float32)
    return acc.reshape(B, S, DIM)
